# revision 12
# baseline (speedup 1.0000x reference)
"""EquiNN kernel for Trainium2 (Bass, raw), 8-core data parallel.

Computes out = l*X + g*rowsum(X) + b for X [4096, 8192] f32.
Shards X row-wise across 8 NeuronCores (512 rows each); l/g/b are baked
into the kernel as immediates at trace time (kernel compiled per call).

Raw Bass (no TileContext): this walrus build allows only one sync-wait
per DMACopy and few on the tail Drain, which Tile's auto-sem assignment
exceeds. With explicit sems every DMA carries 0 waits and every wait is
its own 1-sem instruction; there is also no Tile tail barrier (~10us).

Per-core structure (512x8192 shard = 4 groups of 128 rows):
  SP engine   : 4 HWDGE loads  HBM->SBUF (4 MB each), no waits
  DVE engine  : per group: rowsum, s=g*rs+b, x=l*x+s (in place)
  Pool engine : 4 SWDGE stores SBUF->HBM after group's DVE done
All four groups live in SBUF simultaneously (128 KB/partition) so there
is no buffer reuse and no WAR hazard.
"""

import os

import numpy as np

import concourse.bass as bass
from concourse import mybir
from concourse.bass_utils import run_bass_kernel_spmd

N_CORES = 8
ROWS, COLS = 4096, 8192
SHARD = ROWS // N_CORES  # 512 rows per core
P = 128                  # SBUF partitions
N_GROUPS = SHARD // P    # 4

# Filled in by kernel() when BASS_KERNEL_TRACE=1.
LAST_PROFILE = {}


def _build(l: float, g: float, b: float, reps: int = 1) -> bass.Bass:
    nc = bass.Bass()
    X = nc.declare_dram_parameter("X", [SHARD, COLS], mybir.dt.float32, isOutput=False)
    out = nc.declare_dram_parameter("out", [SHARD, COLS], mybir.dt.float32, isOutput=True)

    Xg = X.rearrange("(gr p) c -> gr p c", p=P)
    outg = out.rearrange("(gr p) c -> gr p c", p=P)

    f32 = mybir.dt.float32
    import contextlib

    with contextlib.ExitStack() as ctx:
        xt = [
            ctx.enter_context(nc.sbuf_tensor(f"xt{i}", [P, COLS], f32))
            for i in range(N_GROUPS)
        ]
        rs = [
            ctx.enter_context(nc.sbuf_tensor(f"rs{i}", [P, 1], f32))
            for i in range(N_GROUPS)
        ]
        s = [
            ctx.enter_context(nc.sbuf_tensor(f"s{i}", [P, 1], f32))
            for i in range(N_GROUPS)
        ]
        load_sems = [
            ctx.enter_context(nc.semaphore(f"load_sem{i}")) for i in range(N_GROUPS)
        ]
        store_sems = [
            ctx.enter_context(nc.semaphore(f"store_sem{i}")) for i in range(N_GROUPS)
        ]
        dve_sem = ctx.enter_context(nc.semaphore("dve_sem"))
        block = ctx.enter_context(nc.Block())

        @block.sync
        def _(sync):
            for r in range(reps):
                for gr in range(N_GROUPS):
                    if r > 0:
                        # next rep may not overwrite xt before store r-1 read it
                        sync.wait_ge(store_sems[gr], 16 * r)
                    sync.dma_start(xt[gr][:], Xg[gr]).then_inc(load_sems[gr], 16)

        @block.vector
        def _(vector):
            for r in range(reps):
                for gr in range(N_GROUPS):
                    k = 3 * (N_GROUPS * r + gr)
                    vector.wait_ge(load_sems[gr], 16 * (r + 1))
                    # DVE is deeply pipelined: back-to-back ops with a data
                    # dep need an explicit same-engine sem wait.
                    nc.vector.reduce_sum(
                        rs[gr][:], xt[gr][:], axis=mybir.AxisListType.X
                    ).then_inc(dve_sem, 1)
                    vector.wait_ge(dve_sem, k + 1)
                    nc.vector.tensor_scalar(
                        s[gr][:], rs[gr][:], g, b,
                        op0=mybir.AluOpType.mult, op1=mybir.AluOpType.add,
                    ).then_inc(dve_sem, 1)
                    vector.wait_ge(dve_sem, k + 2)
                    nc.vector.tensor_scalar(
                        xt[gr][:], xt[gr][:], l, s[gr][:],
                        op0=mybir.AluOpType.mult, op1=mybir.AluOpType.add,
                    ).then_inc(dve_sem, 1)

        @block.gpsimd
        def _(gpsimd):
            for r in range(reps):
                for gr in range(N_GROUPS):
                    gpsimd.wait_ge(dve_sem, 3 * (N_GROUPS * r + gr + 1))
                    gpsimd.dma_start(outg[gr], xt[gr][:]).then_inc(
                        store_sems[gr], 16
                    )
            for gr in range(N_GROUPS):
                gpsimd.wait_ge(store_sems[gr], 16 * reps)

    return nc


def kernel(X: np.ndarray, l: np.ndarray, g: np.ndarray, b: np.ndarray) -> np.ndarray:
    nc = _build(float(l[0]), float(g[0]), float(b[0]))

    shards = np.ascontiguousarray(X, dtype=np.float32).reshape(N_CORES, SHARD, COLS)
    in_maps = [{"X": shards[i]} for i in range(N_CORES)]

    trace = os.environ.get("BASS_KERNEL_TRACE") == "1"
    res = run_bass_kernel_spmd(nc, in_maps, list(range(N_CORES)), trace=trace)
    if trace:
        LAST_PROFILE.update(
            exec_time_ns=res.exec_time_ns,
            mean_exec_time_ns=res.mean_exec_time_ns,
            trace=res.instructions_and_trace[1] if res.instructions_and_trace else None,
            profile_json=res.profile_json,
        )
    return np.concatenate([res.results[i]["out"] for i in range(N_CORES)], axis=0)


# revision 14
# speedup vs baseline: 1.4016x; 1.4016x over previous
"""EquiNN kernel for Trainium2 (Bass, raw), 8-core data parallel.

Computes out = l*X + g*rowsum(X) + b for X [4096, 8192] f32.
Shards X row-wise across 8 NeuronCores (512 rows each); l/g/b are baked
into the kernel as immediates at trace time (kernel compiled per call).

Raw Bass (no TileContext): this walrus build allows only one sync-wait
per DMACopy and few on the tail Drain, which Tile's auto-sem assignment
exceeds. With explicit sems every DMA carries 0 waits and every wait is
its own 1-sem instruction; there is also no Tile tail barrier (~10us).

Per-core structure (512x8192 shard = 4 groups of 128 rows):
  SP engine   : 4 HWDGE loads  HBM->SBUF (4 MB each), no waits
  DVE engine  : per group: rowsum, s=g*rs+b, x=l*x+s (in place)
  Pool engine : 4 SWDGE stores SBUF->HBM after group's DVE done
All four groups live in SBUF simultaneously (128 KB/partition) so there
is no buffer reuse and no WAR hazard.
"""

import os

import numpy as np

import concourse.bass as bass
from concourse import mybir
from concourse.bass_utils import run_bass_kernel_spmd

N_CORES = 8
ROWS, COLS = 4096, 8192
SHARD = ROWS // N_CORES  # 512 rows per core
P = 128                  # SBUF partitions
N_GROUPS = SHARD // P    # 4

# Filled in by kernel() when BASS_KERNEL_TRACE=1.
LAST_PROFILE = {}


def _build(l: float, g: float, b: float, reps: int = 1, mode: str = "full") -> bass.Bass:
    nc = bass.Bass()
    X = nc.declare_dram_parameter("X", [SHARD, COLS], mybir.dt.float32, isOutput=False)
    out = nc.declare_dram_parameter("out", [SHARD, COLS], mybir.dt.float32, isOutput=True)

    Xg = X.rearrange("(gr p) c -> gr p c", p=P)
    outg = out.rearrange("(gr p) c -> gr p c", p=P)

    f32 = mybir.dt.float32
    import contextlib

    with contextlib.ExitStack() as ctx:
        xt = [
            ctx.enter_context(nc.sbuf_tensor(f"xt{i}", [P, COLS], f32))
            for i in range(N_GROUPS)
        ]
        rs = [
            ctx.enter_context(nc.sbuf_tensor(f"rs{i}", [P, 1], f32))
            for i in range(N_GROUPS)
        ]
        s = [
            ctx.enter_context(nc.sbuf_tensor(f"s{i}", [P, 1], f32))
            for i in range(N_GROUPS)
        ]
        load_sems = [
            ctx.enter_context(nc.semaphore(f"load_sem{i}")) for i in range(N_GROUPS)
        ]
        store_sems = [
            ctx.enter_context(nc.semaphore(f"store_sem{i}")) for i in range(N_GROUPS)
        ]
        dve_sem = ctx.enter_context(nc.semaphore("dve_sem"))
        block = ctx.enter_context(nc.Block())

        @block.sync
        def _(sync):
            for r in range(reps):
                for gr in range(N_GROUPS):
                    if r > 0 and mode != "load_only":
                        # next rep may not overwrite xt before store r-1 read it
                        sync.wait_ge(store_sems[gr], 16 * r)
                    sync.dma_start(xt[gr][:], Xg[gr]).then_inc(load_sems[gr], 16)
            if mode == "load_only":
                for gr in range(N_GROUPS):
                    sync.wait_ge(load_sems[gr], 16 * reps)

        if mode == "load_only":
            return nc

        if mode == "copy_only":

            @block.gpsimd
            def _(gpsimd):
                for r in range(reps):
                    for gr in range(N_GROUPS):
                        gpsimd.wait_ge(load_sems[gr], 16 * (r + 1))
                        gpsimd.dma_start(outg[gr], xt[gr][:]).then_inc(
                            store_sems[gr], 16
                        )
                for gr in range(N_GROUPS):
                    gpsimd.wait_ge(store_sems[gr], 16 * reps)

            return nc

        @block.vector
        def _(vector):
            for r in range(reps):
                for gr in range(N_GROUPS):
                    k = 3 * (N_GROUPS * r + gr)
                    vector.wait_ge(load_sems[gr], 16 * (r + 1))
                    # DVE is deeply pipelined: back-to-back ops with a data
                    # dep need an explicit same-engine sem wait.
                    nc.vector.reduce_sum(
                        rs[gr][:], xt[gr][:], axis=mybir.AxisListType.X
                    ).then_inc(dve_sem, 1)
                    vector.wait_ge(dve_sem, k + 1)
                    nc.vector.tensor_scalar(
                        s[gr][:], rs[gr][:], g, b,
                        op0=mybir.AluOpType.mult, op1=mybir.AluOpType.add,
                    ).then_inc(dve_sem, 1)
                    vector.wait_ge(dve_sem, k + 2)
                    nc.vector.tensor_scalar(
                        xt[gr][:], xt[gr][:], l, s[gr][:],
                        op0=mybir.AluOpType.mult, op1=mybir.AluOpType.add,
                    ).then_inc(dve_sem, 1)

        @block.gpsimd
        def _(gpsimd):
            for r in range(reps):
                for gr in range(N_GROUPS):
                    gpsimd.wait_ge(dve_sem, 3 * (N_GROUPS * r + gr + 1))
                    gpsimd.dma_start(outg[gr], xt[gr][:]).then_inc(
                        store_sems[gr], 16
                    )
            for gr in range(N_GROUPS):
                gpsimd.wait_ge(store_sems[gr], 16 * reps)

    return nc


def kernel(X: np.ndarray, l: np.ndarray, g: np.ndarray, b: np.ndarray) -> np.ndarray:
    nc = _build(float(l[0]), float(g[0]), float(b[0]))

    shards = np.ascontiguousarray(X, dtype=np.float32).reshape(N_CORES, SHARD, COLS)
    in_maps = [{"X": shards[i]} for i in range(N_CORES)]

    trace = os.environ.get("BASS_KERNEL_TRACE") == "1"
    res = run_bass_kernel_spmd(nc, in_maps, list(range(N_CORES)), trace=trace)
    if trace:
        LAST_PROFILE.update(
            exec_time_ns=res.exec_time_ns,
            mean_exec_time_ns=res.mean_exec_time_ns,
            trace=res.instructions_and_trace[1] if res.instructions_and_trace else None,
            profile_json=res.profile_json,
        )
    return np.concatenate([res.results[i]["out"] for i in range(N_CORES)], axis=0)


# revision 17
# speedup vs baseline: 1.9611x; 1.3992x over previous
"""EquiNN kernel for Trainium2 (Bass, raw), 8-core data parallel.

Computes out = l*X + g*rowsum(X) + b for X [4096, 8192] f32.
Shards X row-wise across 8 NeuronCores (512 rows each); l/g/b are baked
into the kernel as immediates at trace time (kernel compiled per call).

Raw Bass (no TileContext): this walrus build allows only one sync-wait
per DMACopy and few on the tail Drain, which Tile's auto-sem assignment
exceeds. With explicit sems every DMA carries 0 waits and every wait is
its own 1-sem instruction; there is also no Tile tail barrier (~10us).

Per-core structure (512x8192 shard = 4 groups of 128 rows):
  SP engine   : 4 HWDGE loads  HBM->SBUF (4 MB each), no waits
  DVE engine  : per group: rowsum, s=g*rs+b, x=l*x+s (in place)
  Pool engine : 4 SWDGE stores SBUF->HBM after group's DVE done
All four groups live in SBUF simultaneously (128 KB/partition) so there
is no buffer reuse and no WAR hazard.
"""

import os

import numpy as np

import concourse.bass as bass
from concourse import mybir
from concourse.bass_utils import run_bass_kernel_spmd

N_CORES = 8
ROWS, COLS = 4096, 8192
SHARD = ROWS // N_CORES  # 512 rows per core
P = 128                  # SBUF partitions
N_GROUPS = SHARD // P    # 4

# Filled in by kernel() when BASS_KERNEL_TRACE=1.
LAST_PROFILE = {}


def _build(l: float, g: float, b: float, reps: int = 1, mode: str = "full") -> bass.Bass:
    nc = bass.Bass()
    X = nc.declare_dram_parameter("X", [SHARD, COLS], mybir.dt.float32, isOutput=False)
    out = nc.declare_dram_parameter("out", [SHARD, COLS], mybir.dt.float32, isOutput=True)

    Xg = X.rearrange("(gr p) c -> gr p c", p=P)
    outg = out.rearrange("(gr p) c -> gr p c", p=P)

    f32 = mybir.dt.float32
    import contextlib

    with contextlib.ExitStack() as ctx:
        if mode != "load_only_big":
            xt = [
                ctx.enter_context(nc.sbuf_tensor(f"xt{i}", [P, COLS], f32))
                for i in range(N_GROUPS)
            ]
            rs = [
                ctx.enter_context(nc.sbuf_tensor(f"rs{i}", [P, 1], f32))
                for i in range(N_GROUPS)
            ]
            s = [
                ctx.enter_context(nc.sbuf_tensor(f"s{i}", [P, 1], f32))
                for i in range(N_GROUPS)
            ]
        load_sems = [
            ctx.enter_context(nc.semaphore(f"load_sem{i}")) for i in range(N_GROUPS)
        ]
        store_sems = [
            ctx.enter_context(nc.semaphore(f"store_sem{i}")) for i in range(N_GROUPS)
        ]
        dve_sem = ctx.enter_context(nc.semaphore("dve_sem"))
        block = ctx.enter_context(nc.Block())

        if mode == "load_only_big":
            # single DMA for the whole shard: partition p holds rows
            # {gr*128+p}: [P, N_GROUPS, COLS] in SBUF (128KB/partition)
            Xbig = X.rearrange("(gr p) c -> p gr c", p=P)
            xtb = ctx.enter_context(
                nc.sbuf_tensor("xtb", [P, N_GROUPS, COLS], f32)
            )

            @block.sync
            def _(sync):
                for r in range(reps):
                    sync.dma_start(xtb[:], Xbig).then_inc(load_sems[0], 16)
                sync.wait_ge(load_sems[0], 16 * reps)

            return nc

        if mode == "load_only_sw":

            @block.gpsimd
            def _(gpsimd):
                for r in range(reps):
                    for gr in range(N_GROUPS):
                        gpsimd.dma_start(xt[gr][:], Xg[gr]).then_inc(
                            load_sems[gr], 16
                        )
                for gr in range(N_GROUPS):
                    gpsimd.wait_ge(load_sems[gr], 16 * reps)

            return nc

        if mode == "load_only_2ring":

            @block.sync
            def _(sync):
                for r in range(reps):
                    for gr in range(0, N_GROUPS, 2):
                        sync.dma_start(xt[gr][:], Xg[gr]).then_inc(
                            load_sems[gr], 16
                        )
                for gr in range(0, N_GROUPS, 2):
                    sync.wait_ge(load_sems[gr], 16 * reps)

            @block.scalar
            def _(scalar):
                for r in range(reps):
                    for gr in range(1, N_GROUPS, 2):
                        scalar.dma_start(xt[gr][:], Xg[gr]).then_inc(
                            load_sems[gr], 16
                        )
                for gr in range(1, N_GROUPS, 2):
                    scalar.wait_ge(load_sems[gr], 16 * reps)

            return nc

        @block.sync
        def _(sync):
            for r in range(reps):
                for gr in range(N_GROUPS):
                    if r > 0 and mode != "load_only":
                        # next rep may not overwrite xt before store r-1 read it
                        sync.wait_ge(store_sems[gr], 16 * r)
                    sync.dma_start(xt[gr][:], Xg[gr]).then_inc(load_sems[gr], 16)
            if mode == "load_only":
                for gr in range(N_GROUPS):
                    sync.wait_ge(load_sems[gr], 16 * reps)

        if mode == "load_only":
            return nc

        if mode == "copy_only":

            @block.gpsimd
            def _(gpsimd):
                for r in range(reps):
                    for gr in range(N_GROUPS):
                        gpsimd.wait_ge(load_sems[gr], 16 * (r + 1))
                        gpsimd.dma_start(outg[gr], xt[gr][:]).then_inc(
                            store_sems[gr], 16
                        )
                for gr in range(N_GROUPS):
                    gpsimd.wait_ge(store_sems[gr], 16 * reps)

            return nc

        @block.vector
        def _(vector):
            for r in range(reps):
                for gr in range(N_GROUPS):
                    k = 3 * (N_GROUPS * r + gr)
                    vector.wait_ge(load_sems[gr], 16 * (r + 1))
                    # DVE is deeply pipelined: back-to-back ops with a data
                    # dep need an explicit same-engine sem wait.
                    nc.vector.reduce_sum(
                        rs[gr][:], xt[gr][:], axis=mybir.AxisListType.X
                    ).then_inc(dve_sem, 1)
                    vector.wait_ge(dve_sem, k + 1)
                    nc.vector.tensor_scalar(
                        s[gr][:], rs[gr][:], g, b,
                        op0=mybir.AluOpType.mult, op1=mybir.AluOpType.add,
                    ).then_inc(dve_sem, 1)
                    vector.wait_ge(dve_sem, k + 2)
                    nc.vector.tensor_scalar(
                        xt[gr][:], xt[gr][:], l, s[gr][:],
                        op0=mybir.AluOpType.mult, op1=mybir.AluOpType.add,
                    ).then_inc(dve_sem, 1)

        @block.gpsimd
        def _(gpsimd):
            for r in range(reps):
                for gr in range(N_GROUPS):
                    gpsimd.wait_ge(dve_sem, 3 * (N_GROUPS * r + gr + 1))
                    gpsimd.dma_start(outg[gr], xt[gr][:]).then_inc(
                        store_sems[gr], 16
                    )
            for gr in range(N_GROUPS):
                gpsimd.wait_ge(store_sems[gr], 16 * reps)

    return nc


def kernel(X: np.ndarray, l: np.ndarray, g: np.ndarray, b: np.ndarray) -> np.ndarray:
    nc = _build(float(l[0]), float(g[0]), float(b[0]))

    shards = np.ascontiguousarray(X, dtype=np.float32).reshape(N_CORES, SHARD, COLS)
    in_maps = [{"X": shards[i]} for i in range(N_CORES)]

    trace = os.environ.get("BASS_KERNEL_TRACE") == "1"
    res = run_bass_kernel_spmd(nc, in_maps, list(range(N_CORES)), trace=trace)
    if trace:
        LAST_PROFILE.update(
            exec_time_ns=res.exec_time_ns,
            mean_exec_time_ns=res.mean_exec_time_ns,
            trace=res.instructions_and_trace[1] if res.instructions_and_trace else None,
            profile_json=res.profile_json,
        )
    return np.concatenate([res.results[i]["out"] for i in range(N_CORES)], axis=0)
